# revision 10
# baseline (speedup 1.0000x reference)
"""DinoSAM OC-extractor (slot attention + predictor) Trainium2 Bass kernel.

Sharding: data-parallel over batch B=8 across 8 NeuronCores (one batch
element per core). Params replicated. Full inputs in, full outputs out.

Per-core dataflow (per time step t):
  - HWDGE DMA x_t [1024,768] f32 natural layout.
  - LN(inputs) folded algebraically: per-token mean/rstd via bn_stats;
    rstd is folded into x on-chip (xr = x * rstd, cast to bf16 in the
    same DVE op); mean fixed via rank-1 corrections using mr = m*rstd.
  - xr transposed via XBAR DMA-transpose (SBUF->SBUF, 8x per t) - no
    PE transposes for the big operand.
  - kqT = (W_KQ)^T-stationary pass where W_KQ = (g*Wk*SCALE) @ (g_sl*Wq)^T
    folds the slot-attention q-projection into the k-projection.
  - v in natural layout via x-stationary pass.
  - 3 slot-attention iterations + GRU + MLP + transformer predictor per t,
    fully unrolled, small ops on [7,192]-shaped state; all weight matmuls
    in bf16.
"""

import sys
import os

sys.path.insert(0, "/opt/trn_rl_repo")

import numpy as np

import concourse.bass as bass
import concourse.bacc as bacc
import concourse.mybir as mybir
import concourse.tile as tile
from concourse.bass_utils import run_bass_kernel_spmd
from concourse.masks import make_identity

B, T, N, DENC = 8, 16, 1024, 768
S, D, ITERS, HEADS = 7, 192, 3, 4
HD = D // HEADS
EPS = 1e-8
SCALE = D ** -0.5
LN_EPS = 1e-5

dt = mybir.dt
F32, BF16 = dt.float32, dt.bfloat16
Alu = mybir.AluOpType
Act = mybir.ActivationFunctionType

NTOK = N // 128          # 8 token tiles
NKC = DENC // 128        # 6 contraction chunks
DCH = [(0, 128), (128, 64)]   # d=192 as chunks (offset, size)
FFDIM = 4 * D            # 768


def _dch_slices(tile14):
    """[128,14] tile holding a [192,7] transposed operand: chunk views."""
    return [tile14[0:128, 0:S], tile14[0:64, S:2 * S]]


class _Emitter:
    def __init__(self, ctx, tc):
        self.ctx = ctx
        self.tc = tc
        self.nc = tc.nc

    # ---------- small-op helpers on [S, D]-shaped state ----------

    def ln_stats(self, pool, s_ap):
        """-> (m [S,1], rstd [S,1]) f32."""
        nc = self.nc
        st = pool.tile([S, 6], F32, tag="lnst")
        nc.vector.bn_stats(out=st, in_=s_ap)
        mv = pool.tile([S, 2], F32, tag="lnmv")
        nc.vector.bn_aggr(out=mv, in_=st)
        rs = pool.tile([S, 1], F32, tag="lnrs")
        # sqrt(var + eps)
        nc.scalar.activation(out=rs, in_=mv[:, 1:2], func=Act.Sqrt,
                             bias=self.eps_col[0:S, :])
        nc.vector.reciprocal(out=rs, in_=rs)
        return mv[:, 0:1], rs

    def standardize(self, pool, s_ap, m, rstd, out_dtype=BF16, tag="zs"):
        """(s - m) * rstd -> [S, D] tile."""
        nc = self.nc
        z = pool.tile([S, D], out_dtype, tag=tag)
        nc.vector.tensor_scalar(
            out=z, in0=s_ap, scalar1=m, scalar2=rstd,
            op0=Alu.subtract, op1=Alu.mult,
        )
        return z

    def transpose_sd(self, pools, src, ident, tag="t14", out_dtype=BF16):
        """[S, X<=192] (f32 or bf16) -> [128, 14] tile holding X^T chunks."""
        nc = self.nc
        sbuf, psum = pools
        pt = psum.tile([128, 2 * 8], src.dtype, tag="chps")
        for ci, (off, sz) in enumerate(DCH):
            nc.tensor.matmul(
                pt[0:sz, ci * 8:ci * 8 + S],
                src[:, off:off + sz],
                ident[0:S, 0:S],
                is_transpose=True,
            )
        out = sbuf.tile([128, 2 * S], out_dtype, tag=tag)
        nc.vector.tensor_copy(out[0:128, 0:S], pt[0:128, 0:S])
        nc.vector.tensor_copy(out[0:64, S:2 * S], pt[0:64, 8:8 + S])
        return out

    def evict(self, pool, ps, dtype=BF16, tag="ev", engine=None, scale=None):
        nc = self.nc
        out = pool.tile(list(ps.shape), dtype, tag=tag)
        if scale is not None:
            nc.scalar.activation(out=out, in_=ps, func=Act.Copy, scale=scale)
        elif engine == "act":
            nc.scalar.copy(out=out, in_=ps)
        else:
            nc.vector.tensor_copy(out, ps)
        return out


def build_program(t_steps=T, zero_bias=True):
    assert zero_bias, "nonzero folded biases not implemented"
    nc = bacc.Bacc("TRN2", target_bir_lowering=False, debug=False, num_devices=8)

    x_d = nc.dram_tensor("x", [t_steps, N, DENC], F32, kind="ExternalInput")
    s0_d = nc.dram_tensor("s0", [S, D], F32, kind="ExternalInput")
    wkq_d = nc.dram_tensor("wkq", [DENC, D], BF16, kind="ExternalInput")
    wv_d = nc.dram_tensor("wv", [DENC, D], BF16, kind="ExternalInput")
    nckq_d = nc.dram_tensor("nckq", [D, 1], BF16, kind="ExternalInput")
    ncv_d = nc.dram_tensor("ncv8", [NTOK, NTOK * D], F32, kind="ExternalInput")
    wihT_d = nc.dram_tensor("wihT", [D, 3 * D], F32, kind="ExternalInput")
    whhT_d = nc.dram_tensor("whhT", [D, 3 * D], F32, kind="ExternalInput")
    w1_d = nc.dram_tensor("w1", [D, FFDIM], F32, kind="ExternalInput")
    w2_d = nc.dram_tensor("w2", [FFDIM, D], F32, kind="ExternalInput")
    wqkv_d = nc.dram_tensor("wqkv", [D, 3 * D], F32, kind="ExternalInput")
    wo_d = nc.dram_tensor("wo", [D, D], F32, kind="ExternalInput")
    pw1_d = nc.dram_tensor("pw1", [D, FFDIM], F32, kind="ExternalInput")
    pw2_d = nc.dram_tensor("pw2", [FFDIM, D], F32, kind="ExternalInput")

    osl_d = nc.dram_tensor("out_sl", [t_steps, S, D], F32, kind="ExternalOutput")
    osi_d = nc.dram_tensor("out_si", [t_steps, S, D], F32, kind="ExternalOutput")

    from contextlib import ExitStack

    with tile.TileContext(nc) as tc:
        with ExitStack() as ctx:
            em = _Emitter(ctx, tc)
            _emit(ctx, tc, em, t_steps,
                  x_d, s0_d, wkq_d, wv_d, nckq_d, ncv_d, wihT_d, whhT_d,
                  w1_d, w2_d, wqkv_d, wo_d, pw1_d, pw2_d, osl_d, osi_d)
    nc.compile()
    return nc


def _emit(ctx, tc, em, t_steps, x_d, s0_d, wkq_d, wv_d, nckq_d, ncv_d,
          wihT_d, whhT_d, w1_d, w2_d, wqkv_d, wo_d, pw1_d, pw2_d,
          osl_d, osi_d):
    nc = tc.nc

    singles = ctx.enter_context(tc.tile_pool(name="singles", bufs=1))
    # per-t heavy tiles, double buffered for cross-t overlap
    xap = ctx.enter_context(tc.tile_pool(name="xa", bufs=2))
    xrp = ctx.enter_context(tc.tile_pool(name="xr", bufs=2))
    xtp = ctx.enter_context(tc.tile_pool(name="xt", bufs=2))
    kvp = ctx.enter_context(tc.tile_pool(name="kv", bufs=2))
    stp = ctx.enter_context(tc.tile_pool(name="st", bufs=2))
    # chain scratch
    chp = ctx.enter_context(tc.tile_pool(name="ch", bufs=2))
    # psum pools
    pkq = ctx.enter_context(tc.tile_pool(name="pkq", bufs=2, space="PSUM"))
    pvv = ctx.enter_context(tc.tile_pool(name="pvv", bufs=2, space="PSUM"))
    pch = ctx.enter_context(tc.tile_pool(name="pch", bufs=3, space="PSUM"))

    # identities
    id_bf = singles.tile([128, 128], BF16)
    make_identity(nc, id_bf)
    id_f32 = singles.tile([128, 128], F32)
    make_identity(nc, id_f32)

    # weights to SBUF (chunk-major layouts)
    def load_chunked(name, dram, rows, cols, dtype=BF16):
        nch = (rows + 127) // 128
        t_ = singles.tile([128, nch, cols], dtype, tag=name)
        for c in range(nch):
            sz = min(128, rows - c * 128)
            nc.sync.dma_start(out=t_[0:sz, c, :], in_=dram[c * 128:c * 128 + sz, :])
        return t_

    wkq_sb = load_chunked("wkq", wkq_d, DENC, D)
    wv_sb = load_chunked("wv", wv_d, DENC, D)
    wihT_sb = load_chunked("wihT", wihT_d, D, 3 * D, F32)
    whhT_sb = load_chunked("whhT", whhT_d, D, 3 * D, F32)
    w1_sb = load_chunked("w1", w1_d, D, FFDIM, F32)
    w2_sb = load_chunked("w2", w2_d, FFDIM, D, F32)
    wqkv_sb = load_chunked("wqkv", wqkv_d, D, 3 * D, F32)
    wo_sb = load_chunked("wo", wo_d, D, D, F32)
    pw1_sb = load_chunked("pw1", pw1_d, D, FFDIM, F32)
    pw2_sb = load_chunked("pw2", pw2_d, FFDIM, D, F32)

    nckq_sb = singles.tile([128, 2], BF16)   # neg colsum(W_KQ) as 2 chunks
    for ci, (off, sz) in enumerate(DCH):
        nc.sync.dma_start(out=nckq_sb[0:sz, ci:ci + 1], in_=nckq_d[off:off + sz, :])
    ncv_sb = singles.tile([NTOK, NTOK * D], F32)
    nc.sync.dma_start(out=ncv_sb, in_=ncv_d[:, :])

    s_cur = singles.tile([S, D], F32, tag="slots")
    nc.sync.dma_start(out=s_cur, in_=s0_d[:, :])

    ones_col = singles.tile([128, 1], F32)
    nc.vector.memset(ones_col, 1.0)
    eps_col = singles.tile([128, 1], F32)
    nc.vector.memset(eps_col, LN_EPS)
    em.eps_col = eps_col
    ones_row = singles.tile([1, 128], F32)
    nc.vector.memset(ones_row, 1.0)

    for t in range(t_steps):
        # ---------------- heavy phase: x_t -> stats, xr, xT, kqT, v ---------
        xa = xap.tile([128, NTOK, DENC], F32, tag="xa")  # natural, tok tiles
        nc.sync.dma_start(
            out=xa, in_=x_d[t].rearrange("(j p) d -> p j d", p=128)
        )

        # per-token LN stats (f32)
        mv = stp.tile([128, NTOK, 2], F32, tag="mv")
        for j in range(NTOK):
            bns = stp.tile([128, 2, 6], F32, tag="bns")
            xv = xa[:, j, :].rearrange("p (sg f) -> p sg f", sg=2)
            nc.vector.bn_stats(out=bns[:, 0, :], in_=xv[:, 0, :])
            nc.vector.bn_stats(out=bns[:, 1, :], in_=xv[:, 1, :])
            nc.vector.bn_aggr(out=mv[:, j, :], in_=bns)
        rcol = stp.tile([128, NTOK], F32, tag="rcol")
        nc.scalar.activation(out=rcol, in_=mv[:, :, 1], func=Act.Sqrt, bias=eps_col)
        nc.vector.reciprocal(out=rcol, in_=rcol)
        mcol = mv[:, :, 0]

        # xb = bf16 cast of x (rstd applied downstream: matches reference
        # numerics, which round x before scaling). Split across DVE/GpSimd.
        xb = xrp.tile([128, NTOK, DENC], BF16, tag="xr")
        nc.vector.tensor_copy(xb[:, 0:4, :], xa[:, 0:4, :])
        nc.gpsimd.tensor_copy(xb[:, 4:NTOK, :], xa[:, 4:NTOK, :])

        # transposed means: mT[j, :] = means of token tile j
        mps = pch.tile([NTOK, 128], F32, tag="chps")
        nc.tensor.matmul(mps, mcol, id_f32, is_transpose=True)
        mT = stp.tile([NTOK, 128], F32, tag="mT")
        nc.vector.tensor_copy(mT, mps)

        # x transpose via XBAR dma: xt[p, c, n] = xb[n, c*128+p]
        xt = xtp.tile([128, NKC, N], BF16, tag="xt")
        for j in range(NTOK):
            nc.sync.dma_start(
                out=xt[:, :, j * 128:(j + 1) * 128],
                in_=xb[:, j, :],
                transpose=True,
            )

        # kqT pass (weight-stationary): kqT[d', n] raw (no corrections)
        kqt = kvp.tile([128, 2, N], BF16, tag="kqt")
        for ci, (off, sz) in enumerate(DCH):
            for h in range(2):
                ps = pkq.tile([128, 512], F32, tag="kqps")
                for c in range(NKC):
                    nc.tensor.matmul(
                        ps[0:sz, :],
                        wkq_sb[:, c, off:off + sz],
                        xt[:, c, h * 512:(h + 1) * 512],
                        start=(c == 0), stop=(c == NKC - 1),
                    )
                if (ci + h) % 2 == 0:
                    nc.vector.tensor_copy(kqt[0:sz, ci, h * 512:(h + 1) * 512], ps[0:sz, :])
                else:
                    nc.scalar.copy(out=kqt[0:sz, ci, h * 512:(h + 1) * 512], in_=ps[0:sz, :])

        # v pass (x-stationary) with rank-1 mean fix + rstd eviction scale
        vv = kvp.tile([128, NTOK, D], BF16, tag="vv")
        for j in range(NTOK):
            ps = pvv.tile([128, D], F32, tag="vps")
            for c in range(NKC):
                nc.tensor.matmul(
                    ps, xt[:, c, j * 128:(j + 1) * 128], wv_sb[:, c, :],
                    start=(c == 0), stop=False, skip_group_check=True,
                )
            nc.tensor.matmul(
                ps, mT[0:NTOK, :], ncv_sb[:, j * D:(j + 1) * D],
                start=False, stop=True, skip_group_check=True,
            )
            nc.scalar.activation(out=vv[:, j, :], in_=ps, func=Act.Copy,
                                 scale=rcol[:, j:j + 1])

        # ---------------- recurrent chain for step t ------------------------
        nc.sync.dma_start(out=osi_d[t], in_=s_cur)

        for it in range(ITERS):
            # zs = standardize(slots); zsT
            m_s, r_s = em.ln_stats(chp, s_cur)
            zs = em.standardize(chp, s_cur, m_s, r_s, BF16, tag="zs")
            zsT = em.transpose_sd((chp, pch), zs, id_bf, tag="zsT")
            zsT_sl = _dch_slices(zsT)

            # csn = -colsum_KQ @ zs^T  [1, S]
            csp = pch.tile([1, S], F32, tag="chps")
            for ci, (off, sz) in enumerate(DCH):
                nc.tensor.matmul(
                    csp, nckq_sb[0:sz, ci:ci + 1], zsT_sl[ci],
                    start=(ci == 0), stop=(ci == 1),
                )
            ncsn = chp.tile([1, S], F32, tag="ncsn")
            nc.vector.tensor_copy(ncsn, csp)

            # dots[n, s] per token tile + rank-1 mean fix
            pd = pch.tile([128, NTOK * S], F32, tag="chps")
            for j in range(NTOK):
                sl = pd[:, j * S:(j + 1) * S]
                for ci, (off, sz) in enumerate(DCH):
                    nc.tensor.matmul(
                        sl, kqt[0:sz, ci, j * 128:(j + 1) * 128], zsT_sl[ci],
                        start=(ci == 0), stop=(ci == 1), skip_group_check=True,
                    )
            # mean fix: pd += m (x) ncsn, then scale rows by rstd, in psum
            ncsn_ps = pch.tile([128, S], F32, tag="chps")
            nc.tensor.matmul(ncsn_ps, ones_row, ncsn)
            outer = chp.tile([128, NTOK * S], F32, tag="outer")
            ncsn_b = bass.AP(tensor=ncsn_ps.tensor, offset=ncsn_ps.offset,
                             ap=[ncsn_ps.ap[0], [0, NTOK], ncsn_ps.ap[1]])
            nc.vector.tensor_tensor(
                out=outer.rearrange("p (j s) -> p j s", s=S),
                in0=mcol.to_broadcast([128, NTOK, S]), in1=ncsn_b, op=Alu.mult)
            nc.vector.tensor_tensor(out=pd, in0=pd, in1=outer, op=Alu.add)
            pd3 = pd.rearrange("p (j s) -> p j s", s=S)
            nc.vector.tensor_tensor(
                out=pd3, in0=pd3, in1=rcol.to_broadcast([128, NTOK, S]), op=Alu.mult
            )
            ex = chp.tile([128, NTOK * S], F32, tag="ex")
            nc.scalar.activation(out=ex, in_=pd, func=Act.Exp)

            # softmax over s, +EPS, renorm over n
            ex3 = ex.rearrange("p (j s) -> p j s", s=S)
            rs_ = chp.tile([128, NTOK], F32, tag="rowsum")
            nc.vector.reduce_sum(rs_, ex3, axis=mybir.AxisListType.X)
            nc.vector.reciprocal(out=rs_, in_=rs_)
            at2 = chp.tile([128, NTOK * S], F32, tag="at2")
            at23 = at2.rearrange("p (j s) -> p j s", s=S)
            nc.vector.tensor_tensor(
                out=at23, in0=ex3, in1=rs_.to_broadcast([128, NTOK, S]), op=Alu.mult
            )
            nc.vector.tensor_scalar(
                out=at2, in0=at2, scalar1=EPS, scalar2=None, op0=Alu.add
            )
            # column sums over all tokens: ones^T @ at2 -> [1, NTOK*S]
            csum_ps = pch.tile([1, NTOK * S], F32, tag="chps")
            nc.tensor.matmul(csum_ps, ones_col, at2)
            den = chp.tile([1, S], F32, tag="den")
            csv = bass.AP(
                tensor=csum_ps.tensor, offset=csum_ps.offset,
                ap=[csum_ps.ap[0], [1, S], [S, NTOK]],
            )
            nc.vector.reduce_sum(den, csv, axis=mybir.AxisListType.X)
            nc.vector.reciprocal(out=den, in_=den)
            # physically broadcast den across partitions via K=1 matmul
            den_ps = pch.tile([128, S], F32, tag="chps")
            nc.tensor.matmul(den_ps, ones_row, den)
            attn = chp.tile([128, NTOK * S], BF16, tag="attn")
            attn3 = attn.rearrange("p (j s) -> p j s", s=S)
            den_b = bass.AP(tensor=den_ps.tensor, offset=den_ps.offset,
                            ap=[den_ps.ap[0], [0, NTOK], den_ps.ap[1]])
            nc.vector.tensor_tensor(out=attn3, in0=at2.rearrange(
                "p (j s) -> p j s", s=S), in1=den_b, op=Alu.mult)

            # upd[s, d] = sum_n attn[n, s] v[n, d]  (natural layout)
            pu = pch.tile([S, D], F32, tag="chps")
            for j in range(NTOK):
                nc.tensor.matmul(
                    pu, attn[:, j * S:(j + 1) * S], vv[:, j, :],
                    start=(j == 0), stop=(j == NTOK - 1),
                )
            upd_sb = chp.tile([S, D], F32, tag="upds")
            nc.vector.tensor_copy(upd_sb, pu)
            updT = em.transpose_sd((chp, pch), upd_sb, id_f32, tag="updT",
                                   out_dtype=F32)
            updT_sl = _dch_slices(updT)

            # s^T (prev slots, for GRU h-side)
            sT = em.transpose_sd((chp, pch), s_cur, id_f32, tag="sT",
                                 out_dtype=F32)
            sT_sl = _dch_slices(sT)

            # GRU gates
            psA = pch.tile([S, 2 * D], F32, tag="chps")   # r|z pre-act
            psB = pch.tile([S, D], F32, tag="chps")       # inn
            psC = pch.tile([S, D], F32, tag="chps")       # hn
            for ci, (off, sz) in enumerate(DCH):
                nc.tensor.matmul(psA, updT_sl[ci], wihT_sb[0:sz, ci, 0:2 * D],
                                 start=(ci == 0), stop=False, skip_group_check=True)
                nc.tensor.matmul(psB, updT_sl[ci], wihT_sb[0:sz, ci, 2 * D:3 * D],
                                 start=(ci == 0), stop=(ci == 1))
            for ci, (off, sz) in enumerate(DCH):
                nc.tensor.matmul(psA, sT_sl[ci], whhT_sb[0:sz, ci, 0:2 * D],
                                 start=False, stop=(ci == 1), skip_group_check=True)
                nc.tensor.matmul(psC, sT_sl[ci], whhT_sb[0:sz, ci, 2 * D:3 * D],
                                 start=(ci == 0), stop=(ci == 1))
            rz = chp.tile([S, 2 * D], F32, tag="rz")
            nc.scalar.activation(out=rz, in_=psA, func=Act.Sigmoid)
            rhn = chp.tile([S, D], F32, tag="rhn")
            nc.vector.tensor_tensor(out=rhn, in0=rz[:, 0:D], in1=psC, op=Alu.mult)
            nn_ = chp.tile([S, D], F32, tag="nn")
            nc.vector.tensor_tensor(out=nn_, in0=psB, in1=rhn, op=Alu.add)
            nc.scalar.activation(out=nn_, in_=nn_, func=Act.Tanh)
            # s' = n + z * (h - n)
            hmn = chp.tile([S, D], F32, tag="hmn")
            nc.vector.tensor_tensor(out=hmn, in0=s_cur, in1=nn_, op=Alu.subtract)
            nc.vector.tensor_tensor(out=hmn, in0=hmn, in1=rz[:, D:2 * D], op=Alu.mult)
            s_new = chp.tile([S, D], F32, tag="snew")
            nc.vector.tensor_tensor(out=s_new, in0=nn_, in1=hmn, op=Alu.add)

            if it < ITERS - 1:
                s_new = _ffn_block(em, chp, pch, s_new, w1_sb, w2_sb,
                                   id_bf, id_f32, tag=f"mlp{t}_{it}")
            # copy back into the persistent slots tile
            nc.vector.tensor_copy(s_cur, s_new)

        nc.sync.dma_start(out=osl_d[t], in_=s_cur)

        # ------------- predictor: pre-LN transformer encoder block ----------
        m_p, r_p = em.ln_stats(chp, s_cur)
        zh_p = em.standardize(chp, s_cur, m_p, r_p, F32, tag="pzh")
        zhT = em.transpose_sd((chp, pch), zh_p, id_f32, tag="pzhT", out_dtype=F32)
        zhT_sl = _dch_slices(zhT)

        psqk = pch.tile([S, 2 * D], F32, tag="chps")
        psv = pch.tile([S, D], F32, tag="chps")
        for ci, (off, sz) in enumerate(DCH):
            nc.tensor.matmul(psqk, zhT_sl[ci], wqkv_sb[0:sz, ci, 0:2 * D],
                             start=(ci == 0), stop=(ci == 1))
            nc.tensor.matmul(psv, zhT_sl[ci], wqkv_sb[0:sz, ci, 2 * D:3 * D],
                             start=(ci == 0), stop=(ci == 1))
        qk = em.evict(chp, psqk, F32, tag="qk", engine="act")
        vsb = em.evict(chp, psv, F32, tag="pv", engine="act")

        # per-head transposes of q and k: [S, 48] -> [48, S]
        pqt = pch.tile([48, 2 * HEADS * 8], F32, tag="chps")
        for hh in range(2 * HEADS):
            nc.tensor.matmul(
                pqt[0:48, hh * 8:hh * 8 + S],
                qk[:, hh * HD:(hh + 1) * HD],
                id_f32[0:S, 0:S],
                is_transpose=True,
            )
        qkT = chp.tile([48, 2 * HEADS * S], F32, tag="qkT")
        nc.vector.tensor_copy(
            qkT.rearrange("p (h c) -> p h c", c=S),
            pqt.rearrange("p (h c) -> p h c", c=8)[:, :, 0:S])

        patt = pch.tile([S, HEADS * S], F32, tag="chps")
        for h in range(HEADS):
            nc.tensor.matmul(
                patt[:, h * S:(h + 1) * S],
                qkT[0:48, h * S:(h + 1) * S],
                qkT[0:48, (HEADS + h) * S:(HEADS + h + 1) * S],
            )
        # softmax over last dim (k)
        eatt = chp.tile([S, HEADS * S], F32, tag="eatt")
        nc.scalar.activation(out=eatt, in_=patt, func=Act.Exp)
        eatt3 = eatt.rearrange("p (h s) -> p h s", s=S)
        prs = chp.tile([S, HEADS], F32, tag="prs")
        nc.vector.reduce_sum(prs, eatt3, axis=mybir.AxisListType.X)
        nc.vector.reciprocal(out=prs, in_=prs)
        atts = chp.tile([S, HEADS * S], F32, tag="atts")
        nc.vector.tensor_tensor(
            out=atts.rearrange("p (h s) -> p h s", s=S), in0=eatt3,
            in1=prs.to_broadcast([S, HEADS, S]), op=Alu.mult)

        pattT = pch.tile([S, HEADS * 8], F32, tag="chps")
        for h in range(HEADS):
            nc.tensor.matmul(
                pattT[0:S, h * 8:h * 8 + S],
                atts[:, h * S:(h + 1) * S],
                id_f32[0:S, 0:S],
                is_transpose=True,
            )
        attsT = chp.tile([S, HEADS * S], F32, tag="attsT")
        nc.vector.tensor_copy(
            attsT.rearrange("p (h c) -> p h c", c=S),
            pattT.rearrange("p (h c) -> p h c", c=8)[:, :, 0:S])

        po = pch.tile([S, D], F32, tag="chps")
        for h in range(HEADS):
            nc.tensor.matmul(
                po[:, h * HD:(h + 1) * HD],
                attsT[0:S, h * S:(h + 1) * S],
                vsb[:, h * HD:(h + 1) * HD],
            )
        osb = em.evict(chp, po, F32, tag="osb", engine="act")
        oT = em.transpose_sd((chp, pch), osb, id_f32, tag="oT", out_dtype=F32)
        oT_sl = _dch_slices(oT)
        pso = pch.tile([S, D], F32, tag="chps")
        for ci, (off, sz) in enumerate(DCH):
            nc.tensor.matmul(pso, oT_sl[ci], wo_sb[0:sz, ci, :],
                             start=(ci == 0), stop=(ci == 1))
        s4 = chp.tile([S, D], F32, tag="s4")
        nc.vector.tensor_tensor(out=s4, in0=pso, in1=s_cur, op=Alu.add)

        s5 = _ffn_block(em, chp, pch, s4, pw1_sb, pw2_sb, id_bf, id_f32,
                        tag=f"pffn{t}")
        nc.vector.tensor_copy(s_cur, s5)


def _ffn_block(em, chp, pch, s_in, w1_sb, w2_sb, id_bf, id_f32, tag):
    """s_in + relu(LN(s_in) @ w1) @ w2  (gammas folded into w1 on host)."""
    nc = em.nc
    m_, r_ = em.ln_stats(chp, s_in)
    zh = em.standardize(chp, s_in, m_, r_, F32, tag="ffz")
    zhT = em.transpose_sd((chp, pch), zh, id_f32, tag="ffzT", out_dtype=F32)
    zhT_sl = _dch_slices(zhT)
    ps1a = pch.tile([S, 512], F32, tag="chps")
    ps1b = pch.tile([S, FFDIM - 512], F32, tag="chps")
    for ci, (off, sz) in enumerate(DCH):
        nc.tensor.matmul(ps1a, zhT_sl[ci], w1_sb[0:sz, ci, 0:512],
                         start=(ci == 0), stop=(ci == 1))
        nc.tensor.matmul(ps1b, zhT_sl[ci], w1_sb[0:sz, ci, 512:FFDIM],
                         start=(ci == 0), stop=(ci == 1))
    h1 = chp.tile([S, FFDIM], F32, tag="ffh1")
    nc.scalar.activation(out=h1[:, 0:512], in_=ps1a, func=Act.Relu)
    nc.scalar.activation(out=h1[:, 512:FFDIM], in_=ps1b, func=Act.Relu)
    # transpose h1 [S, 768] -> [128, 6*S]
    ph = pch.tile([128, (FFDIM // 128) * 8], F32, tag="chps")
    for c in range(FFDIM // 128):
        nc.tensor.matmul(
            ph[:, c * 8:c * 8 + S],
            h1[:, c * 128:(c + 1) * 128],
            id_f32[0:S, 0:S],
            is_transpose=True,
        )
    h1T = chp.tile([128, (FFDIM // 128) * S], F32, tag="ffhT")
    nc.vector.tensor_copy(
        h1T.rearrange("p (c k) -> p c k", k=S),
        ph.rearrange("p (c k) -> p c k", k=8)[:, :, 0:S])
    ps2 = pch.tile([S, D], F32, tag="chps")
    for c in range(FFDIM // 128):
        nc.tensor.matmul(ps2, h1T[:, c * S:(c + 1) * S], w2_sb[:, c, :],
                         start=(c == 0), stop=(c == FFDIM // 128 - 1))
    s_out = chp.tile([S, D], F32, tag="ffo")
    nc.vector.tensor_tensor(out=s_out, in0=ps2, in1=s_in, op=Alu.add)
    return s_out


# ------------------------------ host side ---------------------------------

_CACHED = {}


def _get_program(t_steps=T):
    if t_steps not in _CACHED:
        _CACHED[t_steps] = build_program(t_steps)
    return _CACHED[t_steps]


def _ncv8(row):
    out = np.zeros((NTOK, NTOK * D), np.float32)
    for j in range(NTOK):
        out[j, j * D:(j + 1) * D] = row
    return out


def _bf16(a):
    import ml_dtypes
    return np.asarray(a, np.float32).astype(ml_dtypes.bfloat16)


def prepare_maps(inputs, noise, slots_mu, slots_log_sigma, Wq, Wk, Wv,
                 gru_Wih, gru_Whh, gru_bih, gru_bhh, mlp_W1, mlp_b1, mlp_W2,
                 mlp_b2, ln_in_g, ln_in_b, ln_sl_g, ln_sl_b, ln_ff_g, ln_ff_b,
                 p_ln1_g, p_ln1_b, p_Wq, p_bq, p_Wk, p_bk, p_Wv, p_bv, p_Wo,
                 p_bo, p_ln2_g, p_ln2_b, p_W1, p_b1, p_W2, p_b2, t_steps=T):
    f = np.asarray
    zeros = [ln_in_b, ln_sl_b, ln_ff_b, gru_bih, gru_bhh, mlp_b1, mlp_b2,
             p_ln1_b, p_bq, p_bk, p_bv, p_bo, p_ln2_b, p_b1, p_b2]
    for z in zeros:
        assert np.abs(np.asarray(z)).max() == 0.0, "nonzero bias unsupported"

    slots0 = f(slots_mu) + np.exp(f(slots_log_sigma)) * f(noise)  # [B,S,D]

    wk_g = f(ln_in_g)[:, None] * f(Wk) * SCALE          # [768,192]
    wv_g = f(ln_in_g)[:, None] * f(Wv)                  # [768,192]
    wq_g = f(ln_sl_g)[:, None] * f(Wq)                  # [192,192]
    w_kq = wk_g @ wq_g.T                                # [768,192]

    inv_sqrt_hd = 1.0 / np.sqrt(HD)
    wq_p = f(p_ln1_g)[:, None] * f(p_Wq) * inv_sqrt_hd
    wk_p = f(p_ln1_g)[:, None] * f(p_Wk)
    wv_p = f(p_ln1_g)[:, None] * f(p_Wv)
    wqkv = np.concatenate([wq_p, wk_p, wv_p], axis=1)   # [192,576]

    common = {
        "wkq": _bf16(w_kq),
        "wv": _bf16(wv_g),
        "nckq": _bf16(-w_kq.sum(0))[:, None],
        "ncv8": _ncv8(-wv_g.sum(0)),
        "wihT": np.ascontiguousarray(f(gru_Wih).T, np.float32),
        "whhT": np.ascontiguousarray(f(gru_Whh).T, np.float32),
        "w1": np.ascontiguousarray(f(ln_ff_g)[:, None] * f(mlp_W1), np.float32),
        "w2": np.ascontiguousarray(f(mlp_W2), np.float32),
        "wqkv": np.ascontiguousarray(wqkv, np.float32),
        "wo": np.ascontiguousarray(f(p_Wo), np.float32),
        "pw1": np.ascontiguousarray(f(p_ln2_g)[:, None] * f(p_W1), np.float32),
        "pw2": np.ascontiguousarray(f(p_W2), np.float32),
    }
    maps = []
    for b in range(B):
        m = dict(common)
        m["x"] = np.ascontiguousarray(f(inputs)[b, :t_steps], np.float32)
        m["s0"] = np.ascontiguousarray(slots0[b], np.float32)
        maps.append(m)
    return maps


def kernel(**inputs):
    t_steps = T
    nc = _get_program(t_steps)
    maps = prepare_maps(**inputs, t_steps=t_steps)
    res = run_bass_kernel_spmd(nc, maps, core_ids=list(range(B)))
    sl = np.stack([res.results[b]["out_sl"] for b in range(B)])
    si = np.stack([res.results[b]["out_si"] for b in range(B)])
    return sl, si
